# revision 1
# baseline (speedup 1.0000x reference)
"""Trainium2 Bass kernel for nn_Big_MPNN (gnn_message_passing).

Self-contained: hardcodes shapes/sharding. Data-parallel over the batch dim
across 8 NeuronCores (16 graphs per core), weights replicated; no collectives.

Node layout trick: the host sorts nodes by GRU atom-type within each PAIR of
graphs and pads each pair to a fixed 384 columns (per-type capacities uniform
across all pairs/cores, computed from the data at runtime). g is permuted and
zero-padded on the host to match. With that, every per-type GRU matmul reads a
static strided access pattern — no gather/scatter instructions are needed on
the device, and the padded order persists across all 3 message passes.

Per-core dataflow (3 passes), transposed activations [D=128 part, cols]:
  - bond MLP layers 0..6: weights stationary f32r, nodes moving; ReLU on
    ACT/DVE alternating; layer 7 flipped (activation chunks stationary, f16)
    producing normal-layout xb for the aggregation.
  - aggregation m^T = xb^T g^T per (pair, bond) over 3 row-chunks, f16.
  - GRU per (type, branch): 6 matmuls into PSUM over pair-strided segments,
    gates on ACT (sigmoid/tanh) + DVE elementwise.
Output is DMA'd in padded-transposed form; the host unpads/unpermutes.
"""

import numpy as np

import concourse.bass as bass
import concourse.bacc as bacc
import concourse.tile as tile
import concourse.mybir as mybir

F32 = mybir.dt.float32
F32R = mybir.dt.float32r
F16 = mybir.dt.float16
AF = mybir.ActivationFunctionType
ALU = mybir.AluOpType

M = 8                      # cores
B, N, FEAT, D = 128, 128, 75, 128
NB, NL, NT = 7, 8, 6       # bonds, mlp layers, gru type slots
PASSES = 3
BG = B // M                # graphs per core
NPAIR = BG // 2            # graph pairs per core (8)
TOP_ATOMS = [6.0, 7.0, 8.0, 9.0, 0.0]

# dtype knobs
MLP_DT = F32R              # layers 0..6 activations + weights
GRU_X_DT = F32R            # Wih (x-side gru weights)
GRU_M_DT = F32R            # m path: mnT/muT + Whh
AGG_DT = F16               # x7, xb8, g'', W8


def _np_dt(dt):
    return np.float32 if dt in (F32, F32R) else np.float16


def _prepare(g, h, msg_W, gru_Wih, gru_Whh, gru_bih, gru_bhh):
    g = np.ascontiguousarray(np.asarray(g, np.float32))
    h = np.ascontiguousarray(np.asarray(h, np.float32))
    msg_W = np.asarray(msg_W, np.float32)
    gru_Wih = np.asarray(gru_Wih, np.float32).reshape(2, NT, 3, D, D)
    gru_Whh = np.asarray(gru_Whh, np.float32).reshape(2, NT, 3, D, D)
    gru_bih = np.asarray(gru_bih, np.float32).reshape(2, NT, 3, D)
    gru_bhh = np.asarray(gru_bhh, np.float32).reshape(2, NT, 3, D)

    atoms = h[:, :, 0]
    tid = np.full((B, N), NT - 1, np.int32)
    for i, a in enumerate(TOP_ATOMS):
        tid[atoms == np.float32(a)] = i
    tid_pairs = tid.reshape(B // 2, 2 * N)          # all pairs, all cores

    # per-type capacities = max count over all pairs
    counts = np.stack([(tid_pairs == t).sum(axis=1) for t in range(NT)], 1)
    caps = tuple(int(np.ceil(c / 4) * 4) for c in counts.max(axis=0))
    total = sum(caps)
    PP = int(np.ceil(total / 128) * 128)            # padded pair width
    NP = NPAIR * PP                                 # padded per-core cols
    offs = np.cumsum([0] + list(caps))[:-1]         # segment offsets in pair

    # replicated weights, partition-major layouts
    mwT = np.transpose(msg_W, (3, 0, 1, 2))         # [din, k, l, dout]
    mwT06 = np.ascontiguousarray(mwT[:, :, :NL - 1]).astype(_np_dt(MLP_DT))
    mw8T = np.ascontiguousarray(mwT[:, :, NL - 1]).astype(_np_dt(AGG_DT))
    wihT = np.ascontiguousarray(
        np.transpose(gru_Wih, (4, 0, 1, 2, 3))).astype(_np_dt(GRU_X_DT))
    whhT = np.ascontiguousarray(
        np.transpose(gru_Whh, (4, 0, 1, 2, 3))).astype(_np_dt(GRU_M_DT))
    brz = np.ascontiguousarray(
        np.transpose(gru_bih[:, :, :2] + gru_bhh[:, :, :2], (3, 0, 1, 2)))
    binn = np.ascontiguousarray(np.transpose(gru_bih[:, :, 2], (2, 0, 1)))
    bhnn = np.ascontiguousarray(np.transpose(gru_bhh[:, :, 2], (2, 0, 1)))

    h_t = np.concatenate([h, np.zeros((B, N, D - FEAT), np.float32)], axis=2)

    in_maps = []
    placements = []       # per core: padded position of each original node
    for c in range(M):
        pos = np.zeros((BG, N), np.int64)           # padded col per node
        x0 = np.zeros((NP, D), np.float32)
        gP = np.zeros((128, 3 * NPAIR, NB, PP), np.float32)  # [m-part, mchunk*pair, k, n]
        for p in range(NPAIR):
            ga, gb = c * BG + 2 * p, c * BG + 2 * p + 1
            tp = np.concatenate([tid[ga], tid[gb]])            # [256]
            hp = np.concatenate([h_t[ga], h_t[gb]], axis=0)    # [256, D]
            ppos = np.zeros(2 * N, np.int64)
            for t in range(NT):
                idx = np.flatnonzero(tp == t)
                ppos[idx] = p * PP + offs[t] + np.arange(len(idx))
            pos[2 * p] = ppos[:N]
            pos[2 * p + 1] = ppos[N:]
            x0[ppos] = hp
            # padded pair adjacency: gP[m, n] = g[graph, m_orig, n_orig]
            lp = ppos - p * PP                                  # local cols
            for gi, gr in enumerate((ga, gb)):
                li = lp[gi * N:(gi + 1) * N]
                # g[gr, k, n, m] -> block[m_pad, k, n_pad] (transposed)
                blk = np.transpose(g[gr], (2, 0, 1))            # [m, k, n]
                mc, mr = np.divmod(li, 128)
                gP[mr[:, None], (3 * p + mc)[:, None], :, li[None, :]] = \
                    np.transpose(blk, (0, 2, 1))[:, :, :]
        placements.append(pos)
        in_maps.append(dict(
            gP=gP.astype(_np_dt(AGG_DT)),
            x0=np.ascontiguousarray(x0.T).astype(_np_dt(MLP_DT)),
            mwT06=mwT06, mw8T=mw8T, wihT=wihT, whhT=whhT,
            brz=brz, binn=binn, bhnn=bhnn,
        ))
    meta = (caps, PP)
    return in_maps, meta, placements


def _build(meta):
    caps, PP = meta
    NP = NPAIR * PP
    NCH = NP // 128            # 128-col chunks (normal-layout partition chunks)
    nc = bacc.Bacc("TRN2", target_bir_lowering=False, debug=False, num_devices=M)

    gP_d = nc.dram_tensor("gP", [128, 3 * NPAIR, NB, PP], AGG_DT, kind="ExternalInput")
    x0_d = nc.dram_tensor("x0", [128, NP], MLP_DT, kind="ExternalInput")
    mwT06_d = nc.dram_tensor("mwT06", [128, NB, NL - 1, 128], MLP_DT, kind="ExternalInput")
    mw8T_d = nc.dram_tensor("mw8T", [128, NB, 128], AGG_DT, kind="ExternalInput")
    wihT_d = nc.dram_tensor("wihT", [128, 2, NT, 3, 128], GRU_X_DT, kind="ExternalInput")
    whhT_d = nc.dram_tensor("whhT", [128, 2, NT, 3, 128], GRU_M_DT, kind="ExternalInput")
    brz_d = nc.dram_tensor("brz", [128, 2, NT, 2], F32, kind="ExternalInput")
    binn_d = nc.dram_tensor("binn", [128, 2, NT], F32, kind="ExternalInput")
    bhnn_d = nc.dram_tensor("bhnn", [128, 2, NT], F32, kind="ExternalInput")
    y_d = nc.dram_tensor("y", [128, NP], F32, kind="ExternalOutput")

    # gru segment pieces: (type, col-offset-in-pair, n_pairs_start, n_cols)
    pieces = []
    off = 0
    for t in range(NT):
        if caps[t] == 0:
            off += caps[t]
            continue
        per = max(1, min(NPAIR, 512 // caps[t]))
        p0 = 0
        while p0 < NPAIR:
            npr = min(per, NPAIR - p0)
            pieces.append((t, off, p0, npr, caps[t]))
            p0 += npr
        off += caps[t]

    eng_i = [0]

    def copy_engine():
        eng_i[0] += 1
        return nc.scalar if eng_i[0] % 2 == 0 else nc.vector

    with tile.TileContext(nc) as tc:
        with (
            tc.tile_pool(name="const", bufs=1) as cp,
            tc.tile_pool(name="wmlp", bufs=2) as wp,
            tc.tile_pool(name="gpp", bufs=2) as gpp,
            tc.tile_pool(name="xp", bufs=2) as xp,
            tc.tile_pool(name="mlp", bufs=2) as mp,
            tc.tile_pool(name="x7p", bufs=1) as x7p,
            tc.tile_pool(name="xb8p", bufs=NB) as xb8p,
            tc.tile_pool(name="mtp", bufs=1) as mtp,
            tc.tile_pool(name="gates", bufs=12) as gtp,
            tc.tile_pool(name="fin", bufs=1) as fin,
            tc.tile_pool(name="ps", bufs=4, space="PSUM") as psp,
        ):
            mw8T = cp.tile([128, NB, 128], AGG_DT, tag="mw8T")
            wih = cp.tile([128, 2, NT, 3, 128], GRU_X_DT, tag="wih")
            whh = cp.tile([128, 2, NT, 3, 128], GRU_M_DT, tag="whh")
            brz = cp.tile([128, 2, NT, 2], F32, tag="brz")
            binn = cp.tile([128, 2, NT], F32, tag="binn")
            bhnn = cp.tile([128, 2, NT], F32, tag="bhnn")
            nc.sync.dma_start(mw8T[:], mw8T_d.ap())
            nc.sync.dma_start(wih[:], wihT_d.ap())
            nc.sync.dma_start(whh[:], whhT_d.ap())
            nc.sync.dma_start(brz[:], brz_d.ap())
            nc.sync.dma_start(binn[:], binn_d.ap())
            nc.sync.dma_start(bhnn[:], bhnn_d.ap())

            x_cur = xp.tile([128, NP], MLP_DT, tag="x")
            nc.sync.dma_start(x_cur[:], x0_d.ap())

            for p in range(PASSES):
                last = p == PASSES - 1
                # ---- bond MLPs ----
                xb8 = []
                for k in range(NB):
                    mw = wp.tile([128, NL - 1, 128], MLP_DT, tag="mw")
                    nc.sync.dma_start(mw[:], mwT06_d.ap()[:, k])
                    cur = x_cur
                    for l in range(NL - 1):
                        if l == NL - 2:
                            nxt = x7p.tile([128, NP], AGG_DT, tag="x7")
                        else:
                            nxt = mp.tile([128, NP], MLP_DT, tag="mlp")
                        for c2 in range(NP // 1024):
                            ps = psp.tile([128, 1024], F32, tag="ps")
                            for hh in range(2):
                                sl = slice(c2 * 1024 + hh * 512,
                                           c2 * 1024 + (hh + 1) * 512)
                                nc.tensor.matmul(ps[:, hh * 512:(hh + 1) * 512],
                                                 mw[:, l, :], cur[:, sl],
                                                 start=True, stop=True)
                            eng = copy_engine()
                            osl = slice(c2 * 1024, (c2 + 1) * 1024)
                            if eng is nc.scalar:
                                nc.scalar.activation(nxt[:, osl], ps[:], AF.Relu)
                            else:
                                nc.vector.tensor_scalar_max(nxt[:, osl], ps[:], 0.0)
                        cur = nxt
                    # layer 7 flipped -> xb normal [node-chunk part, chunk, dout]
                    xb = xb8p.tile([128, NCH, 128], AGG_DT, tag="xb8")
                    for grp in range(NCH // 4):
                        ps = psp.tile([128, 4, 128], F32, tag="ps")
                        for j in range(4):
                            ci = grp * 4 + j
                            nc.tensor.matmul(ps[:, j, :],
                                             cur[:, ci * 128:(ci + 1) * 128],
                                             mw8T[:, k, :], start=True, stop=True)
                        eng = copy_engine()
                        out_ap = xb[:, grp * 4:(grp + 1) * 4, :]
                        if eng is nc.scalar:
                            nc.scalar.copy(out_ap, ps[:])
                        else:
                            nc.vector.tensor_copy(out_ap, ps[:])
                    xb8.append(xb)

                # ---- aggregation: m^T per pair (3 m-chunks) ----
                mnT = mtp.tile([128, NP], GRU_M_DT, tag="mnT")
                muT = mtp.tile([128, NP], GRU_M_DT, tag="muT")
                for pr in range(NPAIR):
                    ps_n = psp.tile([128, PP], F32, tag="ps")
                    ps_u = psp.tile([128, PP], F32, tag="ps")
                    for mc in range(3):
                        gt = gpp.tile([128, NB, PP], AGG_DT, tag="gt")
                        nc.sync.dma_start(gt[:], gP_d.ap()[:, 3 * pr + mc])
                        for k in range(NB - 1):
                            nc.tensor.matmul(
                                ps_n[:], xb8[k][:, 3 * pr + mc, :],
                                gt[:, k, :],
                                start=(k == 0 and mc == 0),
                                stop=(k == NB - 2 and mc == 2))
                        nc.tensor.matmul(ps_u[:], xb8[NB - 1][:, 3 * pr + mc, :],
                                         gt[:, NB - 1, :],
                                         start=(mc == 0), stop=(mc == 2))
                    osl = slice(pr * PP, (pr + 1) * PP)
                    for ps_t, dst in ((ps_n, mnT), (ps_u, muT)):
                        eng = copy_engine()
                        if eng is nc.scalar:
                            nc.scalar.copy(dst[:, osl], ps_t[:])
                        else:
                            nc.vector.tensor_copy(dst[:, osl], ps_t[:])

                # ---- GRU over type segments (pair-strided APs) ----
                if last:
                    x_next = mp.tile([128, NP], F32, tag="mlp")
                else:
                    x_next = xp.tile([128, NP], MLP_DT, tag="x")
                used = sum(caps)
                if used < PP:
                    for pr in range(NPAIR):
                        nc.vector.memset(
                            x_next[:, pr * PP + used:(pr + 1) * PP].bitcast(F32),
                            0.0)

                def seg(tile_, piece):
                    t, o, p0, npr, w = piece
                    return tile_[:].rearrange("d (pr pp) -> d pr pp", pp=PP)[
                        :, p0:p0 + npr, o:o + w]

                for piece in pieces:
                    t, o, p0, npr, w = piece
                    ncols = npr * w
                    xs_ap = seg(x_cur, piece)
                    hu = []
                    for u in range(2):
                        ms_ap = seg(mnT if u == 0 else muT, piece)
                        ps_rz = psp.tile([128, 2, 512], F32, tag="ps")
                        ps_n2 = psp.tile([128, 2, 512], F32, tag="ps")
                        for gi in range(2):
                            nc.tensor.matmul(ps_rz[:, gi, :ncols], wih[:, u, t, gi, :],
                                             xs_ap, start=True, stop=False)
                            nc.tensor.matmul(ps_rz[:, gi, :ncols], whh[:, u, t, gi, :],
                                             ms_ap, start=False, stop=True)
                        nc.tensor.matmul(ps_n2[:, 0, :ncols], wih[:, u, t, 2, :],
                                         xs_ap, start=True, stop=True)
                        nc.tensor.matmul(ps_n2[:, 1, :ncols], whh[:, u, t, 2, :],
                                         ms_ap, start=True, stop=True)
                        r = gtp.tile([128, 512], F16, tag="gt")
                        z = gtp.tile([128, 512], F16, tag="gt")
                        nc.scalar.activation(r[:, :ncols], ps_rz[:, 0, :ncols],
                                             AF.Sigmoid, bias=brz[:, u, t, 0:1])
                        nc.scalar.activation(z[:, :ncols], ps_rz[:, 1, :ncols],
                                             AF.Sigmoid, bias=brz[:, u, t, 1:2])
                        t1 = gtp.tile([128, 512], F16, tag="gt")
                        nc.vector.scalar_tensor_tensor(
                            t1[:, :ncols], ps_n2[:, 1, :ncols], bhnn[:, u, t:t + 1],
                            r[:, :ncols], op0=ALU.add, op1=ALU.mult)
                        na = gtp.tile([128, 512], F16, tag="gt")
                        nc.vector.scalar_tensor_tensor(
                            na[:, :ncols], ps_n2[:, 0, :ncols], binn[:, u, t:t + 1],
                            t1[:, :ncols], op0=ALU.add, op1=ALU.add)
                        n = gtp.tile([128, 512], F16, tag="gt")
                        nc.scalar.activation(n[:, :ncols], na[:, :ncols], AF.Tanh)
                        d_ = gtp.tile([128, 512], F16, tag="gt")
                        nc.vector.tensor_sub(d_[:, :ncols], ms_ap, n[:, :ncols])
                        e = gtp.tile([128, 512], F16, tag="gt")
                        nc.vector.tensor_mul(e[:, :ncols], z[:, :ncols], d_[:, :ncols])
                        hu_t = gtp.tile([128, 512], F16, tag="gt")
                        nc.vector.tensor_add(hu_t[:, :ncols], n[:, :ncols], e[:, :ncols])
                        hu.append(hu_t)
                    nc.vector.tensor_add(seg(x_next, piece),
                                         hu[0][:, :ncols], hu[1][:, :ncols])
                x_cur = x_next

            nc.sync.dma_start(y_d.ap(), x_cur[:])

    nc.compile()
    return nc


def _make_runner(nc):
    import jax
    from jax.experimental.shard_map import shard_map
    from jax.sharding import Mesh, PartitionSpec, NamedSharding
    from concourse.bass2jax import (install_neuronx_cc_hook, _bass_exec_p,
                                    partition_id_tensor)

    install_neuronx_cc_hook()
    partition_name = (nc.partition_id_tensor.name
                      if nc.partition_id_tensor else None)
    in_names, out_names, out_avals, zero_outs = [], [], [], []
    for alloc in nc.m.functions[0].allocations:
        if not isinstance(alloc, mybir.MemoryLocationSet):
            continue
        name = alloc.memorylocations[0].name
        if alloc.kind == "ExternalInput":
            if name != partition_name:
                in_names.append(name)
        elif alloc.kind == "ExternalOutput":
            out_names.append(name)
            shape = tuple(alloc.tensor_shape)
            dtype = mybir.dt.np(alloc.dtype)
            out_avals.append(jax.core.ShapedArray(shape, dtype))
            zero_outs.append(np.zeros(shape, dtype))
    n_params = len(in_names)
    all_names = in_names + out_names
    if partition_name is not None:
        all_names = all_names + [partition_name]

    def _body(*args):
        operands = list(args)
        if partition_name is not None:
            operands.append(partition_id_tensor())
        outs = _bass_exec_p.bind(
            *operands,
            out_avals=tuple(out_avals),
            in_names=tuple(all_names),
            out_names=tuple(out_names),
            lowering_input_output_aliases=(),
            sim_require_finite=True,
            sim_require_nnan=True,
            nc=nc,
        )
        return tuple(outs)

    devices = jax.devices()[:M]
    mesh = Mesh(np.asarray(devices), ("core",))
    specs = (PartitionSpec("core"),) * (n_params + len(out_names))
    fn = jax.jit(shard_map(_body, mesh=mesh,
                           in_specs=specs,
                           out_specs=(PartitionSpec("core"),) * len(out_names)),
                 keep_unused=True)

    def put(in_maps):
        sh = NamedSharding(mesh, PartitionSpec("core"))
        args = []
        for name in in_names:
            cat = np.concatenate([np.asarray(im[name]) for im in in_maps], axis=0)
            args.append(jax.device_put(cat, sh))
        for z in zero_outs:
            cat = np.concatenate([z] * M, axis=0)
            args.append(jax.device_put(cat, sh))
        return args

    def run(args):
        outs = fn(*args)
        outs = [np.asarray(o) for o in outs]
        per_core = []
        for c in range(M):
            per_core.append({
                name: outs[i].reshape(M, *out_avals[i].shape)[c]
                for i, name in enumerate(out_names)})
        return per_core

    return put, run


_CACHE = {}


def _get_runner(meta):
    if meta not in _CACHE:
        nc = _build(meta)
        _CACHE[meta] = (_make_runner(nc), nc)
    return _CACHE[meta]


def _assemble(per_core, placements):
    out = np.empty((B, N, D), np.float32)
    for c in range(M):
        y = per_core[c]["y"]                      # [D, NP] padded transposed
        pos = placements[c]                       # [BG, N]
        out[c * BG:(c + 1) * BG] = y.T[pos]       # gather real columns
    return out


def kernel(g, h, msg_W, gru_Wih, gru_Whh, gru_bih, gru_bhh):
    in_maps, meta, placements = _prepare(g, h, msg_W, gru_Wih, gru_Whh,
                                         gru_bih, gru_bhh)
    (put, run), _nc = _get_runner(meta)
    args = put(in_maps)
    per_core = run(args)
    return _assemble(per_core, placements)


# exposed for test.py
def get_nc_and_runner(g, h, msg_W, gru_Wih, gru_Whh, gru_bih, gru_bhh):
    in_maps, meta, placements = _prepare(g, h, msg_W, gru_Wih, gru_Whh,
                                         gru_bih, gru_bhh)
    (put, run), nc = _get_runner(meta)
    return in_maps, put, run, nc, placements



# revision 9
# speedup vs baseline: 1.3528x; 1.3528x over previous
"""Trainium2 Bass kernel for nn_Big_MPNN (gnn_message_passing).

Self-contained: hardcodes shapes/sharding. Data-parallel over the batch dim
across 8 NeuronCores (16 graphs per core), weights replicated; no collectives.

Node layout: the host pairs graphs to BALANCE per-type counts (local search
minimizing sum of per-type max counts over pairs), then sorts nodes by GRU
atom-type within each pair. Each pair occupies exactly U = sum(caps) columns
(no dead padding); per-type capacities are uniform across all pairs/cores so
every per-type GRU matmul reads a static strided access pattern.

Per-core dataflow (3 passes), transposed activations [D=128 part, cols],
all f16 except PSUM/biases/final cast:
  - per pair: bond MLP layers 0..6 (bond-interleaved waves, weights resident),
    ReLU drains load-balanced across ACT/DVE/Pool; layer 7 flipped per pair
    (3 chunks: 128/128/rem) producing normal-layout xb; aggregation
    m^T = xb^T g^T accumulated over the 3 row chunks; GRU pieces issued as
    soon as their pair group's aggregation lands.
Output is DMA'd in padded-transposed f16; the host unpads/unpermutes.
"""

import numpy as np

import concourse.bass as bass
import concourse.bacc as bacc
import concourse.tile as tile
import concourse.mybir as mybir

F32 = mybir.dt.float32
F16 = mybir.dt.float16
AF = mybir.ActivationFunctionType
ALU = mybir.AluOpType

M = 8                      # cores
B, N, FEAT, D = 128, 128, 75, 128
NB, NL, NT = 7, 8, 6       # bonds, mlp layers, gru type slots
PASSES = 3
BG = B // M                # graphs per core
NPAIR = BG // 2            # graph pairs per core (8)
TOP_ATOMS = [6.0, 7.0, 8.0, 9.0, 0.0]


def _pair_graphs(cnt):
    """Pair the B graphs to minimize sum_t max_pairs(count_t).  cnt: [B, NT]."""
    P = B // 2
    order = np.argsort(cnt[:, NT - 1], kind="stable")
    pairs = np.stack([order[:P], order[:P - 1:-1]], 1)
    rng = np.random.default_rng(12345)

    def obj(pr):
        pc = cnt[pr[:, 0]] + cnt[pr[:, 1]]
        s = np.sort(pc, 0)[::-1]
        return s[0].sum() * 1000 + s[1].sum() * 10 + s[2].sum()

    cur = pairs.copy()
    co = obj(cur)
    best, bo = cur.copy(), co
    for _ in range(150000):
        i, j = rng.integers(0, P, 2)
        if i == j:
            continue
        trial = cur.copy()
        a1, b1 = trial[i]
        a2, b2 = trial[j]
        if rng.integers(0, 2) == 0:
            trial[i] = (a1, a2)
            trial[j] = (b1, b2)
        else:
            trial[i] = (a1, b2)
            trial[j] = (a2, b1)
        to = obj(trial)
        if to <= co:
            cur, co = trial, to
            if to < bo:
                best, bo = trial.copy(), to
    return best


def _prepare(g, h, msg_W, gru_Wih, gru_Whh, gru_bih, gru_bhh):
    g = np.ascontiguousarray(np.asarray(g, np.float32))
    h = np.ascontiguousarray(np.asarray(h, np.float32))
    msg_W = np.asarray(msg_W, np.float32)
    gru_Wih = np.asarray(gru_Wih, np.float32).reshape(2, NT, 3, D, D)
    gru_Whh = np.asarray(gru_Whh, np.float32).reshape(2, NT, 3, D, D)
    gru_bih = np.asarray(gru_bih, np.float32).reshape(2, NT, 3, D)
    gru_bhh = np.asarray(gru_bhh, np.float32).reshape(2, NT, 3, D)

    atoms = h[:, :, 0]
    tid = np.full((B, N), NT - 1, np.int32)
    for i, a in enumerate(TOP_ATOMS):
        tid[atoms == np.float32(a)] = i
    cnt = np.stack([(tid == t).sum(1) for t in range(NT)], 1).astype(np.int64)

    pairs = _pair_graphs(cnt)                       # [64, 2] graph ids
    pc = cnt[pairs[:, 0]] + cnt[pairs[:, 1]]
    caps = tuple(int(c) for c in pc.max(axis=0))
    U = sum(caps)
    assert 256 < U <= 384, f"caps {caps} sum {U} out of supported range"
    rem = U - 256
    NP = NPAIR * U
    offs = np.cumsum([0] + list(caps))[:-1]

    # replicated weights, partition-major f16 layouts
    mwT = np.transpose(msg_W, (3, 0, 1, 2))         # [din, k, l, dout]
    mwT06 = np.ascontiguousarray(mwT[:, :, :NL - 1]).astype(np.float16)
    mw8T = np.ascontiguousarray(mwT[:, :, NL - 1]).astype(np.float16)
    wihT = np.ascontiguousarray(
        np.transpose(gru_Wih, (4, 0, 1, 2, 3))).astype(np.float16)
    whhT = np.ascontiguousarray(
        np.transpose(gru_Whh, (4, 0, 1, 2, 3))).astype(np.float16)
    brz = np.ascontiguousarray(
        np.transpose(gru_bih[:, :, :2] + gru_bhh[:, :, :2], (3, 0, 1, 2)))
    binn = np.ascontiguousarray(np.transpose(gru_bih[:, :, 2], (2, 0, 1)))
    bhnn = np.ascontiguousarray(np.transpose(gru_bhh[:, :, 2], (2, 0, 1)))

    h_t = np.concatenate([h, np.zeros((B, N, D - FEAT), np.float32)], axis=2)

    in_maps = []
    placements = []     # per core: (gids [BG], pos [BG, N])
    for c in range(M):
        gids = pairs[c * NPAIR:(c + 1) * NPAIR].reshape(-1)
        pos = np.zeros((BG, N), np.int64)
        x0 = np.zeros((NP, D), np.float32)
        gPa = np.zeros((128, NPAIR, 2, NB, U), np.float32)
        gPr = np.zeros((rem, NPAIR, NB, U), np.float32)
        for p in range(NPAIR):
            ga, gb = gids[2 * p], gids[2 * p + 1]
            tp = np.concatenate([tid[ga], tid[gb]])            # [256]
            hp = np.concatenate([h_t[ga], h_t[gb]], axis=0)    # [256, D]
            ppos = np.zeros(2 * N, np.int64)
            for t in range(NT):
                idx = np.flatnonzero(tp == t)
                ppos[idx] = offs[t] + np.arange(len(idx))
            pos[2 * p] = p * U + ppos[:N]
            pos[2 * p + 1] = p * U + ppos[N:]
            x0[p * U + ppos] = hp
            # dense pair block: big[m_row, k, n_col] = g[graph, k, n, m]
            big = np.zeros((U, NB, U), np.float32)
            for gi, gr in enumerate((ga, gb)):
                lg = ppos[gi * N:(gi + 1) * N]
                blk = np.transpose(g[gr], (2, 0, 1))           # [m, k, n]
                big[np.ix_(lg, np.arange(NB), lg)] = blk
            gPa[:, p, 0] = np.transpose(big[:128], (0, 1, 2))
            gPa[:, p, 1] = big[128:256]
            gPr[:, p] = big[256:U]
        placements.append((gids.copy(), pos))
        in_maps.append(dict(
            gPa=gPa.astype(np.float16),
            gPr=gPr.astype(np.float16),
            x0=np.ascontiguousarray(x0.T).astype(np.float16),
            mwT06=mwT06, mw8T=mw8T, wihT=wihT, whhT=whhT,
            brz=brz, binn=binn, bhnn=bhnn,
        ))
    meta = (caps, U)
    return in_maps, meta, placements


class _Balancer:
    """Greedy per-engine load balancer for drain/elementwise ops."""

    def __init__(self, nc):
        self.nc = nc
        self.load = {"A": 0.0, "D": 0.0, "P": 0.0}

    def _cost(self, e, op, cols, psum_src, f16_sbuf):
        if e == "A":
            return cols * 0.8333 + (143.0 if psum_src else 185.0)
        if e == "D":
            if f16_sbuf:
                return cols * 0.521 + 60.0
            return cols * 1.0417 + 125.0
        eff = 0.42 if op in ("add", "sub", "mul") else 0.6
        return cols * 0.8333 / eff + 131.0

    def pick(self, op, cols, psum_src=True, f16_sbuf=False, allow=("A", "D")):
        cand = [(self.load[e] + self._cost(e, op, cols, psum_src, f16_sbuf), e)
                for e in allow]
        _, e = min(cand)
        self.load[e] += self._cost(e, op, cols, psum_src, f16_sbuf)
        return e

    def charge(self, e, op, cols, psum_src=True, f16_sbuf=False):
        self.load[e] += self._cost(e, op, cols, psum_src, f16_sbuf)

    # PSUM sources: GPSIMD has no PSUM access -> ACT/DVE only.
    def relu(self, out, ps, cols):
        e = self.pick("relu", cols)
        if e == "A":
            self.nc.scalar.activation(out, ps, AF.Relu)
        else:
            self.nc.vector.tensor_scalar_max(out, ps, 0.0)

    def copy(self, out, ps, cols):
        e = self.pick("copy", cols)
        if e == "A":
            self.nc.scalar.copy(out, ps)
        else:
            self.nc.vector.tensor_copy(out, ps)

    def stt(self, out, in0, scal, in1, op0, op1, cols):
        self.charge("D", "stt", cols)
        self.nc.vector.scalar_tensor_tensor(out, in0, scal, in1,
                                            op0=op0, op1=op1)

    # SBUF-only f16 elementwise: DVE or Pool.
    def tt(self, op, out, a, b, cols, f16_sbuf=True):
        e = self.pick(op, cols, psum_src=False, f16_sbuf=f16_sbuf,
                      allow=("D", "P"))
        eng = self.nc.vector if e == "D" else self.nc.gpsimd
        getattr(eng, "tensor_" + op)(out, a, b)


def _build(meta):
    caps, U = meta
    rem = U - 256
    NP = NPAIR * U
    nc = bacc.Bacc("TRN2", target_bir_lowering=False, debug=False, num_devices=M)

    gPa_d = nc.dram_tensor("gPa", [128, NPAIR, 2, NB, U], F16, kind="ExternalInput")
    gPr_d = nc.dram_tensor("gPr", [rem, NPAIR, NB, U], F16, kind="ExternalInput")
    x0_d = nc.dram_tensor("x0", [128, NP], F16, kind="ExternalInput")
    mwT06_d = nc.dram_tensor("mwT06", [128, NB, NL - 1, 128], F16, kind="ExternalInput")
    mw8T_d = nc.dram_tensor("mw8T", [128, NB, 128], F16, kind="ExternalInput")
    wih_d = nc.dram_tensor("wihT", [128, 2, NT, 3, 128], F16, kind="ExternalInput")
    whh_d = nc.dram_tensor("whhT", [128, 2, NT, 3, 128], F16, kind="ExternalInput")
    brz_d = nc.dram_tensor("brz", [128, 2, NT, 2], F32, kind="ExternalInput")
    binn_d = nc.dram_tensor("binn", [128, 2, NT], F32, kind="ExternalInput")
    bhnn_d = nc.dram_tensor("bhnn", [128, 2, NT], F32, kind="ExternalInput")
    y_d = nc.dram_tensor("y", [128, NP], F16, kind="ExternalOutput")

    # GRU pieces: (type, col-offset, pair0, n_pairs); issued after pair p0+npr-1
    pieces_at = {pr: [] for pr in range(NPAIR)}
    off = 0
    for t in range(NT):
        if caps[t] == 0:
            continue
        npr = min(4, max(1, 256 // caps[t]))
        while NPAIR % npr:
            npr -= 1
        for p0 in range(0, NPAIR, npr):
            pieces_at[p0 + npr - 1].append((t, off, p0, npr))
        off += caps[t]

    with tile.TileContext(nc) as tc:
        with (
            tc.tile_pool(name="const", bufs=1) as cp,
            tc.tile_pool(name="xp", bufs=2) as xp,
            tc.tile_pool(name="mlp", bufs=16) as mp,
            tc.tile_pool(name="x7p", bufs=8) as x7p,
            tc.tile_pool(name="xbp", bufs=2) as xbp,
            tc.tile_pool(name="gtp", bufs=4) as gtp,
            tc.tile_pool(name="mtp", bufs=1) as mtp,
            tc.tile_pool(name="gates", bufs=24) as ggp,
            tc.tile_pool(name="mps", bufs=2, space="PSUM") as mpsp,
            tc.tile_pool(name="ps", bufs=4, space="PSUM") as psp,
        ):
            bal = _Balancer(nc)

            x_cur = xp.tile([128, NP], F16, tag="x")
            nc.sync.dma_start(x_cur[:], x0_d.ap())
            mwT06 = cp.tile([128, NB, NL - 1, 128], F16, tag="mwT06")
            nc.sync.dma_start(mwT06[:], mwT06_d.ap())

            gtiles = {}
            for pn in (0, 1):
                gta0 = gtp.tile([128, 2, NB, U], F16, tag="gta")
                nc.sync.dma_start(gta0[:], gPa_d.ap()[:, pn])
                gtr0 = gtp.tile([rem, NB, U], F16, tag="gtr")
                nc.sync.dma_start(gtr0[:], gPr_d.ap()[:, pn])
                gtiles[pn] = (gta0, gtr0)

            mw8T = cp.tile([128, NB, 128], F16, tag="mw8T")
            wih = cp.tile([128, 2, NT, 3, 128], F16, tag="wih")
            whh = cp.tile([128, 2, NT, 3, 128], F16, tag="whh")
            brz = cp.tile([128, 2, NT, 2], F32, tag="brz")
            binn = cp.tile([128, 2, NT], F32, tag="binn")
            bhnn = cp.tile([128, 2, NT], F32, tag="bhnn")
            nc.sync.dma_start(mw8T[:], mw8T_d.ap())
            nc.sync.dma_start(wih[:], wih_d.ap())
            nc.sync.dma_start(whh[:], whh_d.ap())
            nc.sync.dma_start(brz[:], brz_d.ap())
            nc.sync.dma_start(binn[:], binn_d.ap())
            nc.sync.dma_start(bhnn[:], bhnn_d.ap())

            def seg(tile_, t_off, p0, npr, w):
                return tile_[:].rearrange("d (pr u) -> d pr u", u=U)[
                    :, p0:p0 + npr, t_off:t_off + w]

            for p in range(PASSES):
                last = p == PASSES - 1
                x_next = xp.tile([128, NP], F16, tag="x")
                mnT = mtp.tile([128, NP], F16, tag="mnT")
                muT = mtp.tile([128, NP], F16, tag="muT")

                for pg in range(NPAIR // 2):
                    prA, prB = 2 * pg, 2 * pg + 1
                    # prefetch next group's adjacency (one group ahead)
                    for pn_ in (prA + 2, prB + 2):
                        pp, pn = (p, pn_) if pn_ < NPAIR else (p + 1, pn_ - NPAIR)
                        if pp < PASSES:
                            gta = gtp.tile([128, 2, NB, U], F16, tag="gta")
                            nc.sync.dma_start(gta[:], gPa_d.ap()[:, pn])
                            gtr = gtp.tile([rem, NB, U], F16, tag="gtr")
                            nc.sync.dma_start(gtr[:], gPr_d.ap()[:, pn])
                            gtiles[pn] = (gta, gtr)

                    # ---- bond MLPs: 2 pairs per PSUM tile, waves over bonds ----
                    curA = [x_cur[:, prA * U:(prA + 1) * U]] * NB
                    curB = [x_cur[:, prB * U:(prB + 1) * U]] * NB
                    x7t = [None] * NB
                    for l in range(NL - 1):
                        oA, oB = [], []
                        for k in range(NB):
                            if l == NL - 2:
                                nt_ = x7p.tile([128, 2, U], F16, tag="x7")
                                x7t[k] = nt_
                            else:
                                nt_ = mp.tile([128, 2, U], F16, tag="mlp")
                            ps = mpsp.tile([128, 2, 512], F32, tag="mps")
                            nc.tensor.matmul(ps[:, 0, :U], mwT06[:, k, l, :],
                                             curA[k], start=True, stop=True)
                            nc.tensor.matmul(ps[:, 1, :U], mwT06[:, k, l, :],
                                             curB[k], start=True, stop=True)
                            bal.relu(nt_[:], ps[:, :, :U], 2 * U)
                            oA.append(nt_[:, 0, :])
                            oB.append(nt_[:, 1, :])
                        curA, curB = oA, oB

                    for i, pr in enumerate((prA, prB)):
                        sl = slice(pr * U, (pr + 1) * U)
                        # ---- layer 7 flipped: chunks 128/128/rem ----
                        xb = xbp.tile([128, NB, 3, 128], F16, tag="xb")
                        for k in range(NB):
                            ps3 = psp.tile([128, 3, 128], F32, tag="ps")
                            nc.tensor.matmul(ps3[:, 0, :], x7t[k][:, i, 0:128],
                                             mw8T[:, k, :], start=True, stop=True)
                            nc.tensor.matmul(ps3[:, 1, :], x7t[k][:, i, 128:256],
                                             mw8T[:, k, :], start=True, stop=True)
                            nc.tensor.matmul(ps3[:rem, 2, :], x7t[k][:, i, 256:U],
                                             mw8T[:, k, :], start=True, stop=True)
                            bal.copy(xb[:, k, 0:2, :], ps3[:, 0:2, :], 256)
                            bal.copy(xb[:rem, k, 2, :], ps3[:rem, 2, :], 128)

                        # ---- aggregation ----
                        gta, gtr = gtiles[pr]
                        ps_n = psp.tile([128, U], F32, tag="ps")
                        ps_u = psp.tile([128, U], F32, tag="ps")
                        for mc in range(2):
                            for k in range(NB - 1):
                                nc.tensor.matmul(ps_n[:], xb[:, k, mc, :],
                                                 gta[:, mc, k, :],
                                                 start=(mc == 0 and k == 0),
                                                 stop=False)
                            nc.tensor.matmul(ps_u[:], xb[:, NB - 1, mc, :],
                                             gta[:, mc, NB - 1, :],
                                             start=(mc == 0), stop=False)
                        for k in range(NB - 1):
                            nc.tensor.matmul(ps_n[:], xb[:rem, k, 2, :],
                                             gtr[:, k, :],
                                             start=False, stop=(k == NB - 2))
                        nc.tensor.matmul(ps_u[:], xb[:rem, NB - 1, 2, :],
                                         gtr[:, NB - 1, :], start=False, stop=True)
                        bal.copy(mnT[:, sl], ps_n[:], U)
                        bal.copy(muT[:, sl], ps_u[:], U)

                        # ---- GRU pieces due at this pair ----
                        for (t, t_off, p0, npr) in pieces_at[pr]:
                            w = caps[t]
                            ncols = npr * w
                            xs_ap = seg(x_cur, t_off, p0, npr, w)
                            hu = []
                            for u in range(2):
                                ms_ap = seg(mnT if u == 0 else muT,
                                            t_off, p0, npr, w)
                                ps_rz = psp.tile([128, 2, 256], F32, tag="ps")
                                ps_n2 = psp.tile([128, 2, 256], F32, tag="ps")
                                for gi in range(2):
                                    nc.tensor.matmul(ps_rz[:, gi, :ncols],
                                                     wih[:, u, t, gi, :], xs_ap,
                                                     start=True, stop=False)
                                    nc.tensor.matmul(ps_rz[:, gi, :ncols],
                                                     whh[:, u, t, gi, :], ms_ap,
                                                     start=False, stop=True)
                                nc.tensor.matmul(ps_n2[:, 0, :ncols],
                                                 wih[:, u, t, 2, :], xs_ap,
                                                 start=True, stop=True)
                                nc.tensor.matmul(ps_n2[:, 1, :ncols],
                                                 whh[:, u, t, 2, :], ms_ap,
                                                 start=True, stop=True)
                                r = ggp.tile([128, 256], F16, tag="gt")
                                z = ggp.tile([128, 256], F16, tag="gt")
                                nc.scalar.activation(r[:, :ncols],
                                                     ps_rz[:, 0, :ncols],
                                                     AF.Sigmoid,
                                                     bias=brz[:, u, t, 0:1])
                                nc.scalar.activation(z[:, :ncols],
                                                     ps_rz[:, 1, :ncols],
                                                     AF.Sigmoid,
                                                     bias=brz[:, u, t, 1:2])
                                bal.charge("A", "act", 2 * ncols)
                                t1 = ggp.tile([128, 256], F16, tag="gt")
                                bal.stt(t1[:, :ncols], ps_n2[:, 1, :ncols],
                                        bhnn[:, u, t:t + 1], r[:, :ncols],
                                        ALU.add, ALU.mult, ncols)
                                na = ggp.tile([128, 256], F16, tag="gt")
                                bal.stt(na[:, :ncols], ps_n2[:, 0, :ncols],
                                        binn[:, u, t:t + 1], t1[:, :ncols],
                                        ALU.add, ALU.add, ncols)
                                n = ggp.tile([128, 256], F16, tag="gt")
                                nc.scalar.activation(n[:, :ncols], na[:, :ncols],
                                                     AF.Tanh)
                                bal.charge("A", "act", ncols, psum_src=False)
                                d_ = ggp.tile([128, 256], F16, tag="gt")
                                bal.tt("sub", d_[:, :ncols], ms_ap,
                                       n[:, :ncols], ncols)
                                e_ = ggp.tile([128, 256], F16, tag="gt")
                                bal.tt("mul", e_[:, :ncols], z[:, :ncols],
                                       d_[:, :ncols], ncols)
                                hu_t = ggp.tile([128, 256], F16, tag="gt")
                                bal.tt("add", hu_t[:, :ncols], n[:, :ncols],
                                       e_[:, :ncols], ncols)
                                hu.append(hu_t)
                            bal.tt("add", seg(x_next, t_off, p0, npr, w),
                                   hu[0][:, :ncols], hu[1][:, :ncols], ncols)

                x_cur = x_next

            nc.sync.dma_start(y_d.ap(), x_cur[:])

    nc.compile()
    return nc


def _make_runner(nc):
    import jax
    from jax.experimental.shard_map import shard_map
    from jax.sharding import Mesh, PartitionSpec, NamedSharding
    from concourse.bass2jax import (install_neuronx_cc_hook, _bass_exec_p,
                                    partition_id_tensor)

    install_neuronx_cc_hook()
    partition_name = (nc.partition_id_tensor.name
                      if nc.partition_id_tensor else None)
    in_names, out_names, out_avals, zero_outs = [], [], [], []
    for alloc in nc.m.functions[0].allocations:
        if not isinstance(alloc, mybir.MemoryLocationSet):
            continue
        name = alloc.memorylocations[0].name
        if alloc.kind == "ExternalInput":
            if name != partition_name:
                in_names.append(name)
        elif alloc.kind == "ExternalOutput":
            out_names.append(name)
            shape = tuple(alloc.tensor_shape)
            dtype = mybir.dt.np(alloc.dtype)
            out_avals.append(jax.core.ShapedArray(shape, dtype))
            zero_outs.append(np.zeros(shape, dtype))
    n_params = len(in_names)
    all_names = in_names + out_names
    if partition_name is not None:
        all_names = all_names + [partition_name]

    def _body(*args):
        operands = list(args)
        if partition_name is not None:
            operands.append(partition_id_tensor())
        outs = _bass_exec_p.bind(
            *operands,
            out_avals=tuple(out_avals),
            in_names=tuple(all_names),
            out_names=tuple(out_names),
            lowering_input_output_aliases=(),
            sim_require_finite=True,
            sim_require_nnan=True,
            nc=nc,
        )
        return tuple(outs)

    devices = jax.devices()[:M]
    mesh = Mesh(np.asarray(devices), ("core",))
    specs = (PartitionSpec("core"),) * (n_params + len(out_names))
    fn = jax.jit(shard_map(_body, mesh=mesh,
                           in_specs=specs,
                           out_specs=(PartitionSpec("core"),) * len(out_names)),
                 keep_unused=True)

    def put(in_maps):
        sh = NamedSharding(mesh, PartitionSpec("core"))
        args = []
        for name in in_names:
            cat = np.concatenate([np.asarray(im[name]) for im in in_maps], axis=0)
            args.append(jax.device_put(cat, sh))
        for z in zero_outs:
            cat = np.concatenate([z] * M, axis=0)
            args.append(jax.device_put(cat, sh))
        return args

    def run(args):
        outs = fn(*args)
        outs = [np.asarray(o) for o in outs]
        per_core = []
        for c in range(M):
            per_core.append({
                name: outs[i].reshape(M, *out_avals[i].shape)[c]
                for i, name in enumerate(out_names)})
        return per_core

    return put, run


_CACHE = {}


def _get_runner(meta):
    if meta not in _CACHE:
        nc = _build(meta)
        _CACHE[meta] = (_make_runner(nc), nc)
    return _CACHE[meta]


def _assemble(per_core, placements):
    out = np.empty((B, N, D), np.float32)
    for c in range(M):
        y = np.asarray(per_core[c]["y"], np.float32)   # [D, NP] padded transposed
        gids, pos = placements[c]
        out[gids] = y.T[pos]
    return out


def kernel(g, h, msg_W, gru_Wih, gru_Whh, gru_bih, gru_bhh):
    in_maps, meta, placements = _prepare(g, h, msg_W, gru_Wih, gru_Whh,
                                         gru_bih, gru_bhh)
    (put, run), _nc = _get_runner(meta)
    args = put(in_maps)
    per_core = run(args)
    return _assemble(per_core, placements)


# exposed for test.py
def get_nc_and_runner(g, h, msg_W, gru_Wih, gru_Whh, gru_bih, gru_bhh):
    in_maps, meta, placements = _prepare(g, h, msg_W, gru_Wih, gru_Whh,
                                         gru_bih, gru_bhh)
    (put, run), nc = _get_runner(meta)
    return in_maps, put, run, nc, placements


# revision 25
# speedup vs baseline: 1.5363x; 1.1356x over previous
"""Trainium2 Bass kernel for nn_Big_MPNN (gnn_message_passing).

Self-contained: hardcodes shapes/sharding. Data-parallel over the batch dim
across 8 NeuronCores (16 graphs per core), weights replicated; no collectives.

Node layout: the host pairs graphs to BALANCE per-type counts (local search
minimizing sum of per-type max counts over pairs), then sorts nodes by GRU
atom-type within each pair. Each pair occupies exactly U = sum(caps) columns
(no dead padding); per-type capacities are uniform across all pairs/cores so
every per-type GRU matmul reads a static strided access pattern.

Per-core dataflow (3 passes), transposed activations [D=128 part, cols],
all f16 except PSUM/biases/final cast:
  - per pair: bond MLP layers 0..6 (bond-interleaved waves, weights resident),
    ReLU drains load-balanced across ACT/DVE/Pool; layer 7 flipped per pair
    (3 chunks: 128/128/rem) producing normal-layout xb; aggregation
    m^T = xb^T g^T accumulated over the 3 row chunks; GRU pieces issued as
    soon as their pair group's aggregation lands.
Output is DMA'd in padded-transposed f16; the host unpads/unpermutes.
"""

import numpy as np

import concourse.bass as bass
import concourse.bacc as bacc
import concourse.tile as tile
import concourse.mybir as mybir

F32 = mybir.dt.float32
F16 = mybir.dt.float16
AF = mybir.ActivationFunctionType
ALU = mybir.AluOpType

M = 8                      # cores
B, N, FEAT, D = 128, 128, 75, 128
NB, NL, NT = 7, 8, 6       # bonds, mlp layers, gru type slots
PASSES = 3
BG = B // M                # graphs per core
NPAIR = BG // 2            # graph pairs per core (8)
TOP_ATOMS = [6.0, 7.0, 8.0, 9.0, 0.0]


def _pair_graphs(cnt):
    """Pair the B graphs to minimize sum_t max_pairs(count_t).  cnt: [B, NT]."""
    P = B // 2
    order = np.argsort(cnt[:, NT - 1], kind="stable")
    pairs = np.stack([order[:P], order[:P - 1:-1]], 1)
    rng = np.random.default_rng(12345)

    def obj(pr):
        pc = cnt[pr[:, 0]] + cnt[pr[:, 1]]
        s = np.sort(pc, 0)[::-1]
        return s[0].sum() * 1000 + s[1].sum() * 10 + s[2].sum()

    cur = pairs.copy()
    co = obj(cur)
    best, bo = cur.copy(), co
    for _ in range(150000):
        i, j = rng.integers(0, P, 2)
        if i == j:
            continue
        trial = cur.copy()
        a1, b1 = trial[i]
        a2, b2 = trial[j]
        if rng.integers(0, 2) == 0:
            trial[i] = (a1, a2)
            trial[j] = (b1, b2)
        else:
            trial[i] = (a1, b2)
            trial[j] = (a2, b1)
        to = obj(trial)
        if to <= co:
            cur, co = trial, to
            if to < bo:
                best, bo = trial.copy(), to
    return best


def _prepare(g, h, msg_W, gru_Wih, gru_Whh, gru_bih, gru_bhh):
    g = np.ascontiguousarray(np.asarray(g, np.float32))
    h = np.ascontiguousarray(np.asarray(h, np.float32))
    msg_W = np.asarray(msg_W, np.float32)
    gru_Wih = np.asarray(gru_Wih, np.float32).reshape(2, NT, 3, D, D)
    gru_Whh = np.asarray(gru_Whh, np.float32).reshape(2, NT, 3, D, D)
    gru_bih = np.asarray(gru_bih, np.float32).reshape(2, NT, 3, D)
    gru_bhh = np.asarray(gru_bhh, np.float32).reshape(2, NT, 3, D)

    atoms = h[:, :, 0]
    tid = np.full((B, N), NT - 1, np.int32)
    for i, a in enumerate(TOP_ATOMS):
        tid[atoms == np.float32(a)] = i
    cnt = np.stack([(tid == t).sum(1) for t in range(NT)], 1).astype(np.int64)

    pairs = _pair_graphs(cnt)                       # [64, 2] graph ids
    pc = cnt[pairs[:, 0]] + cnt[pairs[:, 1]]
    caps = tuple(int(c) for c in pc.max(axis=0))
    U = sum(caps)
    assert 256 < U <= 384, f"caps {caps} sum {U} out of supported range"
    rem = U - 256
    NP = NPAIR * U
    offs = np.cumsum([0] + list(caps))[:-1]

    # replicated weights, partition-major f16 layouts
    mwT = np.transpose(msg_W, (3, 0, 1, 2))         # [din, k, l, dout]
    mwT06 = np.ascontiguousarray(mwT[:, :, :NL - 1]).astype(np.float16)
    mw8T = np.ascontiguousarray(mwT[:, :, NL - 1]).astype(np.float16)
    wihT = np.ascontiguousarray(
        np.transpose(gru_Wih, (4, 0, 1, 2, 3))).astype(np.float16)
    whhT = np.ascontiguousarray(
        np.transpose(gru_Whh, (4, 0, 1, 2, 3))).astype(np.float16)
    brz = np.ascontiguousarray(
        np.transpose(gru_bih[:, :, :2] + gru_bhh[:, :, :2], (3, 0, 1, 2)))
    binn = np.ascontiguousarray(np.transpose(gru_bih[:, :, 2], (2, 0, 1)))
    bhnn = np.ascontiguousarray(np.transpose(gru_bhh[:, :, 2], (2, 0, 1)))

    h_t = np.concatenate([h, np.zeros((B, N, D - FEAT), np.float32)], axis=2)

    in_maps = []
    placements = []     # per core: (gids [BG], pos [BG, N])
    for c in range(M):
        gids = pairs[c * NPAIR:(c + 1) * NPAIR].reshape(-1)
        pos = np.zeros((BG, N), np.int64)
        x0 = np.zeros((NP, D), np.float32)
        gPa = np.zeros((128, NPAIR, 2, NB, U), np.float32)
        gPr = np.zeros((rem, NPAIR, NB, U), np.float32)
        for p in range(NPAIR):
            ga, gb = gids[2 * p], gids[2 * p + 1]
            tp = np.concatenate([tid[ga], tid[gb]])            # [256]
            hp = np.concatenate([h_t[ga], h_t[gb]], axis=0)    # [256, D]
            ppos = np.zeros(2 * N, np.int64)
            for t in range(NT):
                idx = np.flatnonzero(tp == t)
                ppos[idx] = offs[t] + np.arange(len(idx))
            pos[2 * p] = p * U + ppos[:N]
            pos[2 * p + 1] = p * U + ppos[N:]
            x0[p * U + ppos] = hp
            # dense pair block: big[m_row, k, n_col] = g[graph, k, n, m]
            big = np.zeros((U, NB, U), np.float32)
            for gi, gr in enumerate((ga, gb)):
                lg = ppos[gi * N:(gi + 1) * N]
                blk = np.transpose(g[gr], (2, 0, 1))           # [m, k, n]
                big[np.ix_(lg, np.arange(NB), lg)] = blk
            gPa[:, p, 0] = np.transpose(big[:128], (0, 1, 2))
            gPa[:, p, 1] = big[128:256]
            gPr[:, p] = big[256:U]
        placements.append((gids.copy(), pos))
        in_maps.append(dict(
            gPa=gPa.astype(np.float16),
            gPr=gPr.astype(np.float16),
            x0=np.ascontiguousarray(x0.T).astype(np.float16),
            mwT06=mwT06, mw8T=mw8T, wihT=wihT, whhT=whhT,
            brz=brz, binn=binn, bhnn=bhnn,
        ))
    meta = (caps, U)
    return in_maps, meta, placements


class _Balancer:
    """Greedy per-engine load balancer for drain/elementwise ops."""

    def __init__(self, nc):
        self.nc = nc
        self.load = {"A": 0.0, "D": 0.0, "P": 0.0}

    def _cost(self, e, op, cols, psum_src, f16_sbuf):
        if e == "A":
            return cols * 0.8333 + (143.0 if psum_src else 185.0)
        if e == "D":
            if f16_sbuf:
                return cols * 0.521 + 60.0
            return cols * 1.0417 + 125.0
        eff = 0.42 if op in ("add", "sub", "mul") else 0.6
        return cols * 0.8333 / eff + 131.0

    def pick(self, op, cols, psum_src=True, f16_sbuf=False, allow=("A", "D")):
        cand = [(self.load[e] + self._cost(e, op, cols, psum_src, f16_sbuf), e)
                for e in allow]
        _, e = min(cand)
        self.load[e] += self._cost(e, op, cols, psum_src, f16_sbuf)
        return e

    def charge(self, e, op, cols, psum_src=True, f16_sbuf=False):
        self.load[e] += self._cost(e, op, cols, psum_src, f16_sbuf)

    # PSUM sources: GPSIMD has no PSUM access -> ACT/DVE only.
    def relu(self, out, ps, cols):
        e = self.pick("relu", cols)
        if e == "A":
            self.nc.scalar.activation(out, ps, AF.Relu)
        else:
            self.nc.vector.tensor_scalar_max(out, ps, 0.0)

    def copy(self, out, ps, cols):
        e = self.pick("copy", cols)
        if e == "A":
            self.nc.scalar.copy(out, ps)
        else:
            self.nc.vector.tensor_copy(out, ps)

    def stt(self, out, in0, scal, in1, op0, op1, cols):
        self.charge("D", "stt", cols)
        self.nc.vector.scalar_tensor_tensor(out, in0, scal, in1,
                                            op0=op0, op1=op1)

    # SBUF-only f16 elementwise: DVE or Pool.
    def tt(self, op, out, a, b, cols, f16_sbuf=True, allow=("P",)):
        e = self.pick(op, cols, psum_src=False, f16_sbuf=f16_sbuf, allow=allow)
        eng = self.nc.vector if e == "D" else self.nc.gpsimd
        getattr(eng, "tensor_" + op)(out, a, b)


def _build(meta):
    caps, U = meta
    rem = U - 256
    NP = NPAIR * U
    nc = bacc.Bacc("TRN2", target_bir_lowering=False, debug=False, num_devices=M)

    gPa_d = nc.dram_tensor("gPa", [128, NPAIR, 2, NB, U], F16, kind="ExternalInput")
    gPr_d = nc.dram_tensor("gPr", [rem, NPAIR, NB, U], F16, kind="ExternalInput")
    x0_d = nc.dram_tensor("x0", [128, NP], F16, kind="ExternalInput")
    mwT06_d = nc.dram_tensor("mwT06", [128, NB, NL - 1, 128], F16, kind="ExternalInput")
    mw8T_d = nc.dram_tensor("mw8T", [128, NB, 128], F16, kind="ExternalInput")
    wih_d = nc.dram_tensor("wihT", [128, 2, NT, 3, 128], F16, kind="ExternalInput")
    whh_d = nc.dram_tensor("whhT", [128, 2, NT, 3, 128], F16, kind="ExternalInput")
    brz_d = nc.dram_tensor("brz", [128, 2, NT, 2], F32, kind="ExternalInput")
    binn_d = nc.dram_tensor("binn", [128, 2, NT], F32, kind="ExternalInput")
    bhnn_d = nc.dram_tensor("bhnn", [128, 2, NT], F32, kind="ExternalInput")
    y_d = nc.dram_tensor("y", [128, NP], F16, kind="ExternalOutput")

    # GRU pieces: (type, col-offset, pair0, n_pairs); issued after pair p0+npr-1
    pieces_at = {pr: [] for pr in range(NPAIR)}
    off = 0
    for t in range(NT):
        if caps[t] == 0:
            continue
        npr = min(4, max(1, 256 // caps[t]))
        while NPAIR % npr:
            npr -= 1
        for p0 in range(0, NPAIR, npr):
            pieces_at[p0 + npr - 1].append((t, off, p0, npr))
        off += caps[t]

    with tile.TileContext(nc) as tc:
        with (
            tc.tile_pool(name="const", bufs=1) as cp,
            tc.tile_pool(name="xp", bufs=2) as xp,
            tc.tile_pool(name="mlp", bufs=16) as mp,
            tc.tile_pool(name="x7p", bufs=8) as x7p,
            tc.tile_pool(name="xbp", bufs=2) as xbp,
            tc.tile_pool(name="gtp", bufs=4) as gtp,
            tc.tile_pool(name="mtp", bufs=2) as mtp,
            tc.tile_pool(name="gates", bufs=24) as ggp,
            tc.tile_pool(name="mps", bufs=3, space="PSUM") as mpsp,
            tc.tile_pool(name="ps", bufs=2, space="PSUM") as psp,
        ):
            bal = _Balancer(nc)

            x_cur = xp.tile([128, NP], F16, tag="x")
            nc.sync.dma_start(x_cur[:], x0_d.ap())
            mwT06 = cp.tile([128, NB, NL - 1, 128], F16, tag="mwT06")
            nc.sync.dma_start(mwT06[:, :, 0:1, :], mwT06_d.ap()[:, :, 0:1, :])
            nc.sync.dma_start(mwT06[:, :, 1:, :], mwT06_d.ap()[:, :, 1:, :])

            gtiles = {}
            for pn in (0, 1):
                gta0 = gtp.tile([128, 2, NB, U], F16, tag="gta")
                nc.sync.dma_start(gta0[:], gPa_d.ap()[:, pn])
                gtr0 = gtp.tile([rem, NB, U], F16, tag="gtr")
                nc.sync.dma_start(gtr0[:], gPr_d.ap()[:, pn])
                gtiles[pn] = (gta0, gtr0)

            mw8T = cp.tile([128, NB, 128], F16, tag="mw8T")
            wih = cp.tile([128, 2, NT, 3, 128], F16, tag="wih")
            whh = cp.tile([128, 2, NT, 3, 128], F16, tag="whh")
            brz = cp.tile([128, 2, NT, 2], F32, tag="brz")
            binn = cp.tile([128, 2, NT], F32, tag="binn")
            bhnn = cp.tile([128, 2, NT], F32, tag="bhnn")
            nc.sync.dma_start(mw8T[:], mw8T_d.ap())
            nc.sync.dma_start(wih[:], wih_d.ap())
            nc.sync.dma_start(whh[:], whh_d.ap())
            nc.sync.dma_start(brz[:], brz_d.ap())
            nc.sync.dma_start(binn[:], binn_d.ap())
            nc.sync.dma_start(bhnn[:], bhnn_d.ap())

            def seg(tile_, t_off, p0, npr, w):
                return tile_[:].rearrange("d (pr u) -> d pr u", u=U)[
                    :, p0:p0 + npr, t_off:t_off + w]

            def issue_piece(xc, xn, mn, mu, piece, fast_tail):
                t, t_off, p0, npr = piece
                w = caps[t]
                ncols = npr * w
                xs_ap = seg(xc, t_off, p0, npr, w)
                ms = [seg(mn, t_off, p0, npr, w), seg(mu, t_off, p0, npr, w)]
                ps_rz, ps_n2, r, z = [], [], [], []
                tail = ("D", "P") if fast_tail else ("P",)
                for u in range(2):
                    prz = psp.tile([128, 2, 256], F32, tag="ps")
                    pn2 = psp.tile([128, 2, 256], F32, tag="ps")
                    for gi in range(2):
                        nc.tensor.matmul(prz[:, gi, :ncols],
                                         wih[:, u, t, gi, :], xs_ap,
                                         start=True, stop=False)
                        nc.tensor.matmul(prz[:, gi, :ncols],
                                         whh[:, u, t, gi, :], ms[u],
                                         start=False, stop=True)
                    nc.tensor.matmul(pn2[:, 0, :ncols], wih[:, u, t, 2, :],
                                     xs_ap, start=True, stop=True)
                    nc.tensor.matmul(pn2[:, 1, :ncols], whh[:, u, t, 2, :],
                                     ms[u], start=True, stop=True)
                    ps_rz.append(prz)
                    ps_n2.append(pn2)
                for u in range(2):
                    r_ = ggp.tile([128, 256], F16, tag="gt")
                    z_ = ggp.tile([128, 256], F16, tag="gt")
                    nc.scalar.activation(r_[:, :ncols], ps_rz[u][:, 0, :ncols],
                                         AF.Sigmoid, bias=brz[:, u, t, 0:1])
                    nc.scalar.activation(z_[:, :ncols], ps_rz[u][:, 1, :ncols],
                                         AF.Sigmoid, bias=brz[:, u, t, 1:2])
                    bal.charge("A", "act", 2 * ncols)
                    r.append(r_)
                    z.append(z_)
                t1, na, n = [], [], []
                for u in range(2):
                    t1_ = ggp.tile([128, 256], F16, tag="gt")
                    bal.stt(t1_[:, :ncols], ps_n2[u][:, 1, :ncols],
                            bhnn[:, u, t:t + 1], r[u][:, :ncols],
                            ALU.add, ALU.mult, ncols)
                    t1.append(t1_)
                for u in range(2):
                    na_ = ggp.tile([128, 256], F16, tag="gt")
                    bal.stt(na_[:, :ncols], ps_n2[u][:, 0, :ncols],
                            binn[:, u, t:t + 1], t1[u][:, :ncols],
                            ALU.add, ALU.add, ncols)
                    na.append(na_)
                for u in range(2):
                    n_ = ggp.tile([128, 256], F16, tag="gt")
                    nc.scalar.activation(n_[:, :ncols], na[u][:, :ncols],
                                         AF.Tanh)
                    bal.charge("A", "act", ncols, psum_src=False)
                    n.append(n_)
                d_ = []
                for u in range(2):
                    dd = ggp.tile([128, 256], F16, tag="gt")
                    bal.tt("sub", dd[:, :ncols], ms[u], n[u][:, :ncols],
                           ncols, allow=tail)
                    d_.append(dd)
                e_ = []
                for u in range(2):
                    ee = ggp.tile([128, 256], F16, tag="gt")
                    bal.tt("mul", ee[:, :ncols], z[u][:, :ncols],
                           d_[u][:, :ncols], ncols, allow=tail)
                    e_.append(ee)
                hu = []
                for u in range(2):
                    hh = ggp.tile([128, 256], F16, tag="gt")
                    bal.tt("add", hh[:, :ncols], n[u][:, :ncols],
                           e_[u][:, :ncols], ncols, allow=tail)
                    hu.append(hh)
                bal.tt("add", seg(xn, t_off, p0, npr, w),
                       hu[0][:, :ncols], hu[1][:, :ncols], ncols, allow=tail)

            pending = []
            for p in range(PASSES):
                last = p == PASSES - 1
                x_next = xp.tile([128, NP], F16, tag="x")
                mnT = mtp.tile([128, NP], F16, tag="mnT")
                muT = mtp.tile([128, NP], F16, tag="muT")

                for pg in range(NPAIR // 2):
                    prA, prB = 2 * pg, 2 * pg + 1
                    # prefetch next group's adjacency (one group ahead)
                    for pn_ in (prA + 2, prB + 2):
                        pp, pn = (p, pn_) if pn_ < NPAIR else (p + 1, pn_ - NPAIR)
                        if pp < PASSES:
                            gta = gtp.tile([128, 2, NB, U], F16, tag="gta")
                            nc.sync.dma_start(gta[:], gPa_d.ap()[:, pn])
                            gtr = gtp.tile([rem, NB, U], F16, tag="gtr")
                            nc.sync.dma_start(gtr[:], gPr_d.ap()[:, pn])
                            gtiles[pn] = (gta, gtr)

                    # ---- bond MLPs: 2 pairs per PSUM tile, waves over bonds ----
                    curA = [x_cur[:, prA * U:(prA + 1) * U]] * NB
                    curB = [x_cur[:, prB * U:(prB + 1) * U]] * NB
                    x7t = [None] * NB
                    for l in range(NL - 1):
                        oA, oB = [], []
                        for k in range(NB):
                            if l == NL - 2:
                                nt_ = x7p.tile([128, 2, U], F16, tag="x7")
                                x7t[k] = nt_
                            else:
                                nt_ = mp.tile([128, 2, U], F16, tag="mlp")
                            ps = mpsp.tile([128, 2, 512], F32, tag="mps")
                            nc.tensor.matmul(ps[:, 0, :U], mwT06[:, k, l, :],
                                             curA[k], start=True, stop=True)
                            nc.tensor.matmul(ps[:, 1, :U], mwT06[:, k, l, :],
                                             curB[k], start=True, stop=True)
                            bal.relu(nt_[:], ps[:, :, :U], 2 * U)
                            oA.append(nt_[:, 0, :])
                            oB.append(nt_[:, 1, :])
                        curA, curB = oA, oB
                        if l == 0:
                            # deferred GRU pieces interleave with this group's
                            # MLP so drains don't queue behind blocked gates
                            for args in pending:
                                issue_piece(*args, False)
                            pending = []

                    for i, pr in enumerate((prA, prB)):
                        sl = slice(pr * U, (pr + 1) * U)
                        # ---- layer 7 flipped: chunks 128/128/rem ----
                        xb = xbp.tile([128, NB, 3, 128], F16, tag="xb")
                        for k in range(NB):
                            ps3 = mpsp.tile([128, 3, 128], F32, tag="mps")
                            nc.tensor.matmul(ps3[:, 0, :], x7t[k][:, i, 0:128],
                                             mw8T[:, k, :], start=True, stop=True)
                            nc.tensor.matmul(ps3[:, 1, :], x7t[k][:, i, 128:256],
                                             mw8T[:, k, :], start=True, stop=True)
                            nc.tensor.matmul(ps3[:rem, 2, :], x7t[k][:, i, 256:U],
                                             mw8T[:, k, :], start=True, stop=True)
                            bal.copy(xb[:, k, :, :], ps3[:], 384)

                        # ---- aggregation ----
                        gta, gtr = gtiles[pr]
                        ps_n = psp.tile([128, U], F32, tag="ps")
                        ps_u = psp.tile([128, U], F32, tag="ps")
                        for mc in range(2):
                            for k in range(NB - 1):
                                nc.tensor.matmul(ps_n[:], xb[:, k, mc, :],
                                                 gta[:, mc, k, :],
                                                 start=(mc == 0 and k == 0),
                                                 stop=False)
                            nc.tensor.matmul(ps_u[:], xb[:, NB - 1, mc, :],
                                             gta[:, mc, NB - 1, :],
                                             start=(mc == 0), stop=False)
                        for k in range(NB - 1):
                            nc.tensor.matmul(ps_n[:], xb[:rem, k, 2, :],
                                             gtr[:, k, :],
                                             start=False, stop=(k == NB - 2))
                        nc.tensor.matmul(ps_u[:], xb[:rem, NB - 1, 2, :],
                                         gtr[:, NB - 1, :], start=False, stop=True)
                        bal.copy(mnT[:, sl], ps_n[:], U)
                        bal.copy(muT[:, sl], ps_u[:], U)

                        # ---- queue GRU pieces due at this pair ----
                        for piece in pieces_at[pr]:
                            pending.append((x_cur, x_next, mnT, muT, piece))

                    if last and pg == 2:
                        # pairs 0-3 finalized by the pieces flushed above
                        nc.sync.dma_start(y_d.ap()[:, 0:4 * U],
                                          x_next[:, 0:4 * U])

                x_cur = x_next

            for args in pending:
                issue_piece(*args, True)
            nc.sync.dma_start(y_d.ap()[:, 4 * U:], x_cur[:, 4 * U:])

    nc.compile()
    return nc


def _make_runner(nc):
    import jax
    from jax.experimental.shard_map import shard_map
    from jax.sharding import Mesh, PartitionSpec, NamedSharding
    from concourse.bass2jax import (install_neuronx_cc_hook, _bass_exec_p,
                                    partition_id_tensor)

    install_neuronx_cc_hook()
    partition_name = (nc.partition_id_tensor.name
                      if nc.partition_id_tensor else None)
    in_names, out_names, out_avals, zero_outs = [], [], [], []
    for alloc in nc.m.functions[0].allocations:
        if not isinstance(alloc, mybir.MemoryLocationSet):
            continue
        name = alloc.memorylocations[0].name
        if alloc.kind == "ExternalInput":
            if name != partition_name:
                in_names.append(name)
        elif alloc.kind == "ExternalOutput":
            out_names.append(name)
            shape = tuple(alloc.tensor_shape)
            dtype = mybir.dt.np(alloc.dtype)
            out_avals.append(jax.core.ShapedArray(shape, dtype))
            zero_outs.append(np.zeros(shape, dtype))
    n_params = len(in_names)
    all_names = in_names + out_names
    if partition_name is not None:
        all_names = all_names + [partition_name]

    def _body(*args):
        operands = list(args)
        if partition_name is not None:
            operands.append(partition_id_tensor())
        outs = _bass_exec_p.bind(
            *operands,
            out_avals=tuple(out_avals),
            in_names=tuple(all_names),
            out_names=tuple(out_names),
            lowering_input_output_aliases=(),
            sim_require_finite=True,
            sim_require_nnan=True,
            nc=nc,
        )
        return tuple(outs)

    devices = jax.devices()[:M]
    mesh = Mesh(np.asarray(devices), ("core",))
    specs = (PartitionSpec("core"),) * (n_params + len(out_names))
    fn = jax.jit(shard_map(_body, mesh=mesh,
                           in_specs=specs,
                           out_specs=(PartitionSpec("core"),) * len(out_names)),
                 keep_unused=True)

    def put(in_maps):
        sh = NamedSharding(mesh, PartitionSpec("core"))
        args = []
        for name in in_names:
            cat = np.concatenate([np.asarray(im[name]) for im in in_maps], axis=0)
            args.append(jax.device_put(cat, sh))
        for z in zero_outs:
            cat = np.concatenate([z] * M, axis=0)
            args.append(jax.device_put(cat, sh))
        return args

    def run(args):
        outs = fn(*args)
        outs = [np.asarray(o) for o in outs]
        per_core = []
        for c in range(M):
            per_core.append({
                name: outs[i].reshape(M, *out_avals[i].shape)[c]
                for i, name in enumerate(out_names)})
        return per_core

    return put, run


_CACHE = {}


def _get_runner(meta):
    if meta not in _CACHE:
        nc = _build(meta)
        _CACHE[meta] = (_make_runner(nc), nc)
    return _CACHE[meta]


def _assemble(per_core, placements):
    out = np.empty((B, N, D), np.float32)
    for c in range(M):
        y = np.asarray(per_core[c]["y"], np.float32)   # [D, NP] padded transposed
        gids, pos = placements[c]
        out[gids] = y.T[pos]
    return out


def kernel(g, h, msg_W, gru_Wih, gru_Whh, gru_bih, gru_bhh):
    in_maps, meta, placements = _prepare(g, h, msg_W, gru_Wih, gru_Whh,
                                         gru_bih, gru_bhh)
    (put, run), _nc = _get_runner(meta)
    args = put(in_maps)
    per_core = run(args)
    return _assemble(per_core, placements)


# exposed for test.py
def get_nc_and_runner(g, h, msg_W, gru_Wih, gru_Whh, gru_bih, gru_bhh):
    in_maps, meta, placements = _prepare(g, h, msg_W, gru_Wih, gru_Whh,
                                         gru_bih, gru_bhh)
    (put, run), nc = _get_runner(meta)
    return in_maps, put, run, nc, placements
